# revision 1
# baseline (speedup 1.0000x reference)
"""Entity-resolution head on 8 TRN2 NeuronCores.

Pure data-parallel: batch dim (256) is split 32/core; the MLP weights are
replicated.  Each core gathers only the bert rows its spans touch
(indirect DMA), folds first/last/mean span features into one masked
matmul per span side, then runs the small MLP stack with activations kept
transposed (features-on-partitions) as the stationary matmul operand and
weights streamed as the moving operand.
"""

import numpy as np

import concourse.bass as bass
import concourse.mybir as mybir
import concourse.tile as tile
from concourse.bass_utils import run_bass_kernel_spmd
from concourse.masks import make_identity

B, S, H = 256, 512, 1024
HH, LH, NOUT = 512, 512, 3
EPS = 1e-5
NCORES = 8
BC = B // NCORES          # 32 batches per core
LSPAN = 15                # max span length (reference: 1..15)
KROWS = BC * LSPAN        # 480 gathered rows per span side
KPAD = 512                # padded to 4 chunks of 128
NCH = KPAD // 128         # 4
F32 = mybir.dt.float32
import os as _os
USE_F32R = _os.environ.get("KERNEL_F32R", "0") == "1"
F32R = mybir.dt.float32r if USE_F32R else mybir.dt.float32
I32 = mybir.dt.int32

WEIGHT_SPECS = [
    ("Wp1", [H, H]), ("bp1", [H]), ("gp", [H]), ("betap", [H]),
    ("Wp2", [H, HH]), ("bp2", [HH]),
    ("We1", [6 * H, H]), ("be1", [H]), ("ge", [H]), ("betae", [H]),
    ("We2", [H, HH]), ("be2", [HH]),
    ("Wl", [2 * HH, LH]), ("bl", [LH]),
    ("Wc", [LH, NOUT]), ("bc", [NOUT]),
]


def _bcast_rows(ap, p):
    """AP view of a 1-D DRAM tensor broadcast across p partitions."""
    return bass.AP(tensor=ap.tensor, offset=ap.offset, ap=[[0, p]] + list(ap.ap))


def _build_program():
    nc = bass.Bass()

    bert = nc.declare_dram_parameter("bert", [BC, S, H], F32, isOutput=False)
    idxA = nc.declare_dram_parameter("idxA", [128, NCH], I32, isOutput=False)
    idxB = nc.declare_dram_parameter("idxB", [128, NCH], I32, isOutput=False)
    idxP = nc.declare_dram_parameter("idxP", [BC, 1], I32, isOutput=False)
    MA = nc.declare_dram_parameter("MA", [128, NCH, 3 * BC], F32R, isOutput=False)
    MB = nc.declare_dram_parameter("MB", [128, NCH, 3 * BC], F32R, isOutput=False)
    w = {}
    _R = {"Wp1", "Wp2", "We1", "We2", "Wl"}
    for name, shape in WEIGHT_SPECS:
        w[name] = nc.declare_dram_parameter(
            name, shape, F32R if name in _R else F32, isOutput=False)
    out = nc.declare_dram_parameter("out", [BC, NOUT], F32, isOutput=True)

    bert2d = bert[:].rearrange("b s h -> (b s) h")   # [16384, H], offset 0

    with tile.TileContext(nc) as tc:
        with (
            tc.tile_pool(name="singles", bufs=1) as singles,
            tc.tile_pool(name="wstream", bufs=6) as wstream,
            tc.tile_pool(name="acts", bufs=1) as acts,
            tc.tile_pool(name="pbig", bufs=1, space="PSUM") as pbig,
            tc.tile_pool(name="pshare", bufs=3, space="PSUM") as pshare,
            tc.tile_pool(name="pdummy", bufs=1, space="PSUM") as pdummy,
        ):
            # ---- constants / small inputs -------------------------------
            ident32 = singles.tile([32, 32], F32, tag="ident32")
            make_identity(nc, ident32[:])
            ident96 = singles.tile([96, 96], F32, tag="ident96")
            make_identity(nc, ident96[:])
            eps_t = singles.tile([BC, 1], F32, tag="eps")
            nc.vector.memset(eps_t[:], EPS)

            # Walrus on this toolchain allows exactly ONE sync-wait per
            # instruction.  pe_observe() is a throwaway 32x32 transpose that
            # makes the PE observe one fresh semaphore so real matmuls only
            # ever need a single wait.  All observers accumulate into ONE
            # psum tile as a single matmul group so they never create
            # PSUM WAR hazards (which would need a second wait).
            N_OBSERVERS = 4
            dummy_ps = pdummy.tile([32, 32], F32, tag="dummy")
            obs_count = [0]

            def pe_observe(src_ap, name):
                i = obs_count[0]
                obs_count[0] += 1
                nc.tensor.matmul(
                    dummy_ps[:], lhsT=src_ap, rhs=ident32[:],
                    is_transpose=True,
                    start=(i == 0), stop=(i == N_OBSERVERS - 1),
                    skip_group_check=True)

            pe_observe(ident96[0:32, 0:32], "ident")

            # Same single-wait rule applies to DMA-queue instructions: a
            # recycled weight slot would need waits on the prior loads' lane
            # sems (WAW) and on the PE readers (WAR).  Before reusing a
            # slot, spend one sync-queue nop per outstanding semaphore so
            # the recycled load itself only carries its own-lane wait.
            from concourse.tile import add_dep_helper

            def _raw(inst):
                return inst.ins if hasattr(inst, "ins") else inst

            def engine_absorb(eng, *dep_insts):
                deps = [d for d in dep_insts if d is not None]
                if not deps:
                    return None
                dr = None
                for d in deps:
                    dr = eng.drain(fusable=False)
                    add_dep_helper(_raw(dr), _raw(d), sync=True,
                                   reason="engine observes producer")
                return dr

            def order_after(inst, dr):
                if dr is not None and inst is not None:
                    add_dep_helper(_raw(inst), _raw(dr), sync=False,
                                   reason="consumer ordered after absorber")

            def sync_absorb(*dep_insts):
                return engine_absorb(nc.sync, *dep_insts)

            wt_hist = []          # FIFO of (load_insts, last_mm_inst)

            ia = singles.tile([128, NCH], I32, tag="ia")
            nc.gpsimd.dma_start(ia[:], idxA[:])
            ib = singles.tile([128, NCH], I32, tag="ib")
            nc.gpsimd.dma_start(ib[:], idxB[:])
            ip = singles.tile([BC, 1], I32, tag="ip")
            nc.gpsimd.dma_start(ip[:], idxP[:])

            ma = singles.tile([128, NCH, 3 * BC], F32R, tag="ma")
            nc.gpsimd.dma_start(ma[:], MA[:])
            mb = singles.tile([128, NCH, 3 * BC], F32R, tag="mb")
            nc.gpsimd.dma_start(mb[:], MB[:])
            pe_observe(ma[0:32, 0, 0:32].bitcast(F32), "ma")
            pe_observe(mb[0:32, 0, 0:32].bitcast(F32), "mb")

            # replicated bias / norm-param rows
            rep = {}
            for name in ("bp1", "gp", "betap", "be1", "ge", "betae",
                         "bp2", "be2", "bl", "bc"):
                n = w[name].shape[0]
                t = singles.tile([BC, n], F32, tag=f"rep_{name}")
                nc.gpsimd.dma_start(t[:], _bcast_rows(w[name][:], BC))
                rep[name] = t
            # absorb each broadcast's DMA-lane semaphore into the DVE clock
            dve_scratch = singles.tile([1, 16], F32, tag="dve_scratch")
            for i, name in enumerate(rep):
                nc.vector.tensor_copy(dve_scratch[0:1, i:i + 1],
                                      rep[name][0:1, 0:1])

            # ---- gathers ------------------------------------------------
            def gather_span(idx_tile, tag):
                tiles = []
                for c in range(NCH):
                    g = singles.tile([128, H], F32R, tag=f"{tag}{c}")
                    nc.gpsimd.indirect_dma_start(
                        out=g[:], out_offset=None,
                        in_=bert2d,
                        in_offset=bass.IndirectOffsetOnAxis(
                            ap=idx_tile[:, c:c + 1], axis=0),
                    )
                    tiles.append(g)
                return tiles

            GA = gather_span(ia, "ga")
            GB = gather_span(ib, "gb")
            GP = singles.tile([BC, H], F32, tag="gp_rows")
            nc.gpsimd.indirect_dma_start(
                out=GP[:], out_offset=None, in_=bert2d,
                in_offset=bass.IndirectOffsetOnAxis(ap=ip[:, 0:1], axis=0),
            )

            # ---- span features: S = M.T @ G  -> [96, H] -----------------
            def span_feats(m_tile, g_tiles, tag):
                ps = [pshare.tile([96, 512], F32, tag="share", name=f"ps_{tag}{h}")
                      for h in range(2)]
                for c in range(NCH):
                    for h in range(2):
                        nc.tensor.matmul(
                            ps[h][:],
                            lhsT=m_tile[:, c, :],
                            rhs=g_tiles[c][:, h * 512:(h + 1) * 512],
                            start=(c == 0), stop=(c == NCH - 1),
                        )
                sb = singles.tile([96, H], F32, tag=f"sf_{tag}")
                for h in range(2):
                    nc.vector.tensor_copy(sb[:, h * 512:(h + 1) * 512], ps[h][:])
                return sb

            SA = span_feats(ma, GA, "a")
            SB = span_feats(mb, GB, "b")

            # transpose span feats -> [128, 8, 96] per side
            def transpose_feats(src, tag):
                dst = singles.tile([128, 8, 96], F32R, tag=f"t_{tag}")
                cp = None
                for h in range(8):
                    pt = pshare.tile([128, 96], F32, tag="share", name="pt96")
                    nc.tensor.transpose(
                        pt[:], src[:, h * 128:(h + 1) * 128], ident96[:])
                    cp = nc.vector.tensor_copy(dst[:, h, :], pt[:])
                return dst, cp

            AT, AT_cp = transpose_feats(SA, "a")
            BT, BT_cp = transpose_feats(SB, "b")

            # pron rows transposed -> [128, 8, 32]
            pe_observe(GP[0:32, 0:32], "gp_lane")
            PT = singles.tile([128, 8, BC], F32R, tag="ptron")
            PT_cp = None
            for h in range(8):
                pt = pshare.tile([128, 96], F32, tag="share", name="pt32")
                pt = pt[:, :BC]
                nc.tensor.transpose(
                    pt[:], GP[:, h * 128:(h + 1) * 128], ident32[:])
                PT_cp = nc.vector.tensor_copy(PT[:, h, :], pt[:])

            # transpose a batch-major [BC, n*128] activation -> [128, n, BC]
            def transpose_act(src, n, tag, dt=F32R):
                dst = acts.tile([128, n, BC], dt, tag=f"tact_{tag}")
                cp = None
                for h in range(n):
                    pt = pshare.tile([128, 96], F32, tag="share", name="pt32")
                    pt = pt[:, :BC]
                    nc.tensor.transpose(
                        pt[:], src[:, h * 128:(h + 1) * 128], ident32[:])
                    cp = nc.vector.tensor_copy(dst[:, h, :], pt[:])
                return dst, cp

            # layer-1 style matmul: act_T chunks [128, BC] x W [K, N] -> psum
            stream_state = {"last_mm": None}

            def stream_matmul(psum_ap, lhsT_chunks, w_dram, ktiles, n_out,
                              tag, lhsT_deps=()):
                for k in range(ktiles):
                    dr_s = None
                    if len(wt_hist) >= 6:
                        old_loads, old_mm = wt_hist.pop(0)
                        dr_s = sync_absorb(old_mm, *old_loads)
                    wt = wstream.tile([128, n_out], F32R, tag="wtile")
                    loads = []
                    for h in range(0, n_out, 512):
                        hi = min(h + 512, n_out)
                        # ≤2KB per partition per DMA keeps each load on one
                        # HWDGE queue -> single lane wait for consumers
                        ld = nc.sync.dma_start(
                            wt[:, h:hi],
                            w_dram[k * 128:(k + 1) * 128, h:hi])
                        order_after(ld, dr_s)
                        loads.append(ld)
                    dr_e = None
                    if k == 0:
                        dr_e = engine_absorb(nc.tensor, *lhsT_deps, *loads,
                                             stream_state["last_mm"])
                    mm = None
                    for h in range(0, n_out, 512):
                        hi = min(h + 512, n_out)
                        mm = nc.tensor.matmul(
                            psum_ap[:, h:hi],
                            lhsT=lhsT_chunks(k),
                            rhs=wt[:, h:hi],
                            start=(k == 0), stop=(k == ktiles - 1),
                        )
                        order_after(mm, dr_e)
                    wt_hist.append((loads, mm))
                stream_state["last_mm"] = mm

            # LayerNorm + affine + leaky-relu epilogue (batch-major [BC, n])
            def ln_leaky(psum_t, bias_t, g_t, beta_t, n, tag):
                x = acts.tile([BC, n], F32, tag=f"ln_{tag}")
                nc.vector.tensor_add(x[:], psum_t[:], bias_t[:])
                nsub = n // 512
                stats = acts.tile([BC, nsub, 6], F32, tag=f"st_{tag}")
                xv = x[:].rearrange("p (s f) -> p s f", f=512)
                for s in range(nsub):
                    nc.vector.bn_stats(out=stats[:, s, :], in_=xv[:, s, :])
                mv = acts.tile([BC, 2], F32, tag=f"mv_{tag}")
                nc.vector.bn_aggr(out=mv[:], in_=stats[:])
                std = acts.tile([BC, 1], F32, tag=f"sd_{tag}")
                nc.scalar.activation(
                    out=std[:], in_=mv[:, 1:2],
                    func=mybir.ActivationFunctionType.Sqrt,
                    bias=eps_t[:], scale=1.0)
                rstd = acts.tile([BC, 1], F32, tag=f"rs_{tag}")
                nc.vector.reciprocal(out=rstd[:], in_=std[:])
                nc.vector.tensor_scalar(
                    out=x[:], in0=x[:], scalar1=mv[:, 0:1], scalar2=rstd[:],
                    op0=mybir.AluOpType.subtract, op1=mybir.AluOpType.mult)
                nc.vector.tensor_mul(x[:], x[:], g_t[:])
                nc.vector.tensor_add(x[:], x[:], beta_t[:])
                # leaky relu: max(x,0) + 0.01*min(x,0)
                pos = acts.tile([BC, n], F32, tag=f"lp_{tag}")
                nc.vector.tensor_scalar_max(pos[:], x[:], 0.0)
                nc.vector.tensor_scalar(
                    out=x[:], in0=x[:], scalar1=0.0, scalar2=0.01,
                    op0=mybir.AluOpType.min, op1=mybir.AluOpType.mult)
                nc.vector.tensor_add(x[:], x[:], pos[:])
                return x

            # ---- pron branch layer 1 -----------------------------------
            ps1p = pbig.tile([BC, H], F32, tag="psA", name="ps1p")
            stream_matmul(ps1p, lambda k: PT[:, k, :], w["Wp1"][:], 8, H, "l1p",
                          lhsT_deps=(PT_cp,))
            X1p = ln_leaky(ps1p, rep["bp1"], rep["gp"], rep["betap"], H, "p")

            # ---- ent branch layer 1 ------------------------------------
            def ent_chunk(k):
                blk, h = divmod(k, 8)
                side = AT if blk < 3 else BT
                b = blk % 3
                return side[:, h, b * 32:(b + 1) * 32]

            ps1e = pbig.tile([BC, H], F32, tag="psB", name="ps1e")
            stream_matmul(ps1e, ent_chunk, w["We1"][:], 48, H, "l1e",
                          lhsT_deps=(AT_cp, BT_cp))
            X1e = ln_leaky(ps1e, rep["be1"], rep["ge"], rep["betae"], H, "e")

            X1pT, X1pT_cp = transpose_act(X1p, 8, "x1p")
            X1eT, X1eT_cp = transpose_act(X1e, 8, "x1e")

            # ---- layer 2 (both branches into one concat tile) ----------
            ps2 = pbig.tile([BC, 2 * HH], F32, tag="psA", name="ps2")
            stream_matmul(ps2[:, 0:HH], lambda k: X1pT[:, k, :],
                          w["Wp2"][:], 8, HH, "l2p", lhsT_deps=(X1pT_cp,))
            stream_matmul(ps2[:, HH:2 * HH], lambda k: X1eT[:, k, :],
                          w["We2"][:], 8, HH, "l2e", lhsT_deps=(X1eT_cp,))
            XC = acts.tile([BC, 2 * HH], F32, tag="xc")
            nc.vector.tensor_add(XC[:, 0:HH], ps2[:, 0:HH], rep["bp2"][:])
            nc.vector.tensor_add(XC[:, HH:], ps2[:, HH:], rep["be2"][:])

            XCT, XCT_cp = transpose_act(XC, 8, "xc")

            # ---- final hidden + exact gelu -----------------------------
            ps3 = pshare.tile([BC, LH], F32, tag="share", name="ps3")
            stream_matmul(ps3, lambda k: XCT[:, k, :], w["Wl"][:], 8, LH,
                          "l3", lhsT_deps=(XCT_cp,))
            g = acts.tile([BC, LH], F32, tag="g")
            g_add = nc.vector.tensor_add(g[:], ps3[:], rep["bl"][:])
            erf = acts.tile([BC, LH], F32, tag="erf")
            erf_act = nc.scalar.activation(
                out=erf[:], in_=g[:],
                func=mybir.ActivationFunctionType.Erf,
                bias=0.0, scale=float(1.0 / np.sqrt(2.0)))
            ge_t = acts.tile([BC, LH], F32, tag="ge_t")
            dr_g = engine_absorb(nc.vector, g_add, erf_act)
            gm = nc.vector.tensor_mul(ge_t[:], g[:], erf[:])
            order_after(gm, dr_g)
            nc.vector.tensor_add(ge_t[:], ge_t[:], g[:])
            nc.vector.tensor_scalar_mul(ge_t[:], ge_t[:], 0.5)

            GT, GT_cp = transpose_act(ge_t, 4, "gt", dt=F32)

            # ---- logits -------------------------------------------------
            ps4 = pshare.tile([BC, NOUT], F32, tag="share", name="ps4")
            wc_loads = []
            wc_tiles = []
            for k in range(4):
                wt = wstream.tile([128, NOUT], F32, tag="wctile")
                wc_tiles.append(wt)
                wc_loads.append(nc.gpsimd.dma_start(
                    wt[:], w["Wc"][k * 128:(k + 1) * 128, :]))
            dr_wc = engine_absorb(nc.tensor, GT_cp, *wc_loads,
                                  stream_state["last_mm"])
            for k in range(4):
                mm = nc.tensor.matmul(
                    ps4[:], lhsT=GT[:, k, :], rhs=wc_tiles[k][:],
                    start=(k == 0), stop=(k == 3))
                order_after(mm, dr_wc)
            res = acts.tile([BC, NOUT], F32, tag="res")
            res_add = nc.vector.tensor_add(res[:], ps4[:], rep["bc"][:])
            sync_absorb(res_add)
            nc.sync.dma_start(out[:], res[:])

    import os
    if not os.environ.get('SKIP_PRUNE'):
        _prune_covered_waits(nc)
    nc.finalize()
    return nc


def _prune_covered_waits(nc):
    """Walrus on this toolchain accepts only one sync-wait on most
    instructions (Drain accepts many).  Within a basic block, same-engine
    instructions execute in order, so a wait already issued by an earlier
    same-engine instruction (e.g. an absorber drain) is redundant on a
    later one and can be dropped."""
    # Split any remaining multi-wait Drain into a chain of 1-wait drains
    # (walrus allows a single sync-wait there too).
    for fn in nc.m.functions:
        for blk in fn.blocks:
            insert = []
            for pos, inst in enumerate(blk.instructions):
                si = inst.sync_info
                if (inst.opcode == "Drain" and si and si.on_wait
                        and len(si.on_wait) > 1):
                    extra = list(si.on_wait[:-1])
                    si.on_wait = [si.on_wait[-1]]
                    insert.append((pos, inst, extra))
            for pos, inst, extra in reversed(insert):
                new_insts = []
                for w in extra:
                    d = mybir.InstDrain(
                        name=nc.get_next_instruction_name(),
                        ins=[], outs=[], bass_is_fusable=False)
                    d.engine = inst.engine
                    d.sync_info = mybir.SyncInfo(on_wait=[w], on_update=[])
                    nc.register_instruction(d)
                    new_insts.append(d)
                blk.instructions[pos:pos] = new_insts

    PRUNABLE = ("DMAHW", "DMASW", "PE_", "DVE_", "Pool_", "Activation_",
                "SP_")

    def prunable(w):
        return (getattr(w, "wait_mode", None) == "sem-ge-imm"
                and w.ant_name.startswith(PRUNABLE))

    for fn in nc.m.functions:
        for blk in fn.blocks:
            observed = {}
            for inst in blk.instructions:
                si = inst.sync_info
                if not si or not si.on_wait:
                    continue
                eng = str(inst.engine)
                kept = []
                for w in si.on_wait:
                    if (prunable(w)
                            and observed.get((eng, w.ant_name), -1)
                            >= w.wait_value):
                        continue
                    kept.append(w)
                for w in si.on_wait:
                    key = (eng, w.ant_name)
                    if prunable(w):
                        if observed.get(key, -1) < w.wait_value:
                            observed[key] = w.wait_value
                if len(kept) != len(si.on_wait):
                    si.on_wait = kept


_PROGRAM = None


def _get_program():
    global _PROGRAM
    if _PROGRAM is None:
        _PROGRAM = _build_program()
    return _PROGRAM


def make_in_maps(**inputs):
    """Shard full inputs into per-core input maps (host-side descriptor prep)."""
    bert = np.ascontiguousarray(np.asarray(inputs["bert_outputs"], dtype=np.float32))
    offsets = np.asarray(inputs["offsets"], dtype=np.int32)
    weights = {name: np.ascontiguousarray(np.asarray(inputs[name], dtype=np.float32))
               for name, _ in WEIGHT_SPECS}

    in_maps = []
    for c in range(NCORES):
        ob = offsets[c * BC:(c + 1) * BC]
        m = {"bert": bert[c * BC:(c + 1) * BC]}

        def span_desc(s, e):
            ln = (e - s).astype(np.int64)          # [BC], 1..15
            j = np.arange(LSPAN)
            rows = (np.arange(BC) * S)[:, None] + s[:, None] + j[None, :]
            idx = np.zeros(KPAD, np.int32)
            idx[:KROWS] = rows.reshape(-1)
            M = np.zeros((KPAD, 3 * BC), np.float32)
            for b in range(BC):
                base = b * LSPAN
                M[base, b] = 1.0                          # first
                M[base + ln[b] - 1, BC + b] = 1.0         # last
                M[base:base + ln[b], 2 * BC + b] = 1.0 / ln[b]  # mean
            return (idx.reshape(NCH, 128).T.copy(),
                    np.ascontiguousarray(
                        M.reshape(NCH, 128, 3 * BC).transpose(1, 0, 2)))

        m["idxA"], m["MA"] = span_desc(ob[:, 0], ob[:, 1])
        m["idxB"], m["MB"] = span_desc(ob[:, 2], ob[:, 3])
        m["idxP"] = (np.arange(BC, dtype=np.int32) * S
                     + ob[:, 4]).reshape(BC, 1)
        m.update(weights)
        in_maps.append(m)
    return in_maps


def run(in_maps, **kwargs):
    nc = _get_program()
    return run_bass_kernel_spmd(nc, in_maps, core_ids=list(range(NCORES)), **kwargs)


def kernel(**inputs):
    res = run(make_in_maps(**inputs))
    return np.concatenate([res.results[c]["out"] for c in range(NCORES)],
                          axis=0).astype(np.float32)



# revision 9
# speedup vs baseline: 1.6941x; 1.6941x over previous
"""Entity-resolution head on 8 TRN2 NeuronCores.

Pure data-parallel: batch dim (256) is split 32/core; the MLP weights are
replicated.  Everything that moves bulk bytes (bert rows, masks, weights,
matmul operands) is bf16; accumulation stays fp32 in PSUM and the LN /
gelu epilogues run fp32 on the vector engine.  Weights are pre-transposed
on the host into the exact SBUF layout and preloaded at kernel start via
~70 DMAs alternating between the two HWDGE queues (sync + scalar), in
consumption order, so the PE streams matmuls while weights arrive.
"""

import numpy as np
import ml_dtypes

import concourse.bass as bass
import concourse.mybir as mybir
import concourse.tile as tile
from concourse.bass_utils import run_bass_kernel_spmd
from concourse.masks import make_identity

B, S, H = 256, 512, 1024
HH, LH, NOUT = 512, 512, 3
EPS = 1e-5
NCORES = 8
BC = B // NCORES          # 32 batches per core
LSPAN = 15                # max span length (reference: 1..15)
KROWS = BC * LSPAN        # 480 gathered rows per span side
KPAD = 512                # padded to 4 chunks of 128
NCH = KPAD // 128         # 4
F32 = mybir.dt.float32
BF16 = mybir.dt.bfloat16
I32 = mybir.dt.int32

# weight matrices streamed into matmuls: (name, K, N, k-chunks per DMA)
MAT_SPECS = [
    ("Wp1", H, H, 1),
    ("We1", 6 * H, H, 1),
    ("Wp2", H, HH, 2),
    ("We2", H, HH, 2),
    ("Wl", 2 * HH, LH, 2),
    ("Wc", LH, NOUT, 4),
]
# biases / norm params packed into one [8, 1024] f32 tensor, bcast to BC rows
BIAS_ROWS = [("bp1",), ("gp",), ("betap",), ("be1",), ("ge",), ("betae",),
             ("bp2", "be2"), ("bl", "bc")]


def _bcast_rows(ap, p):
    """AP view of DRAM tensor broadcast across p partitions."""
    return bass.AP(tensor=ap.tensor, offset=ap.offset, ap=[[0, p]] + list(ap.ap))


def _build_program():
    nc = bass.Bass()

    bert = nc.declare_dram_parameter("bert", [BC, S, H], BF16, isOutput=False)
    idxs = nc.declare_dram_parameter("idxs", [128, 9], I32, isOutput=False)
    masks = nc.declare_dram_parameter("masks", [128, 2, NCH, 3 * BC], BF16,
                                      isOutput=False)
    biases = nc.declare_dram_parameter("biases", [8, 1024], BF16, isOutput=False)
    wd = {}
    for name, K, N, _ in MAT_SPECS:
        wd[name] = nc.declare_dram_parameter(
            f"{name}T", [128, K // 128, N], BF16, isOutput=False)
    out = nc.declare_dram_parameter("out", [BC, NOUT], F32, isOutput=True)

    bert2d = bert[:].rearrange("b s h -> (b s) h")   # [16384, H] bf16

    with tile.TileContext(nc) as tc:
        with (
            tc.tile_pool(name="singles", bufs=1) as singles,
            tc.tile_pool(name="acts", bufs=1) as acts,
            tc.tile_pool(name="pbig", bufs=1, space="PSUM") as pbig,
            tc.tile_pool(name="pshare", bufs=3, space="PSUM") as pshare,
            tc.tile_pool(name="pdummy", bufs=1, space="PSUM") as pdummy,
        ):
            # ---- constants ---------------------------------------------
            ident32 = singles.tile([32, 32], BF16, tag="ident32")
            make_identity(nc, ident32[:])
            ident96 = singles.tile([96, 96], BF16, tag="ident96")
            make_identity(nc, ident96[:])
            eps_t = singles.tile([BC, 1], F32, tag="eps")
            nc.vector.memset(eps_t[:], EPS)

            # Walrus on this toolchain allows exactly ONE sync-wait per
            # instruction.  pe_observe() is a throwaway 32x32 transpose that
            # makes the PE observe one fresh semaphore so real matmuls only
            # ever need a single wait.  All observers accumulate into ONE
            # psum tile as a single matmul group so they never create
            # PSUM WAR hazards (which would need a second wait).
            N_OBSERVERS = 4
            dummy_ps = pdummy.tile([32, 32], BF16, tag="dummy")
            obs_count = [0]

            def pe_observe(src_ap, name):
                i = obs_count[0]
                obs_count[0] += 1
                nc.tensor.matmul(
                    dummy_ps[:], lhsT=src_ap, rhs=ident32[:],
                    is_transpose=True,
                    start=(i == 0), stop=(i == N_OBSERVERS - 1),
                    skip_group_check=True)

            pe_observe(ident96[0:32, 0:32], "ident")

            from concourse.tile import add_dep_helper

            def _raw(inst):
                return inst.ins if hasattr(inst, "ins") else inst

            def engine_absorb(eng, *dep_insts):
                deps = [d for d in dep_insts if d is not None]
                if not deps:
                    return None
                dr = None
                for d in deps:
                    dr = eng.drain(fusable=False)
                    add_dep_helper(_raw(dr), _raw(d), sync=True,
                                   reason="engine observes producer")
                return dr

            def order_after(inst, dr):
                if dr is not None and inst is not None:
                    add_dep_helper(_raw(inst), _raw(dr), sync=False,
                                   reason="consumer ordered after absorber")

            # ---- small SWDGE inputs ------------------------------------
            idx = singles.tile([128, 9], I32, tag="idx")
            nc.gpsimd.dma_start(idx[:], idxs[:])
            mk = singles.tile([128, 2, NCH, 3 * BC], BF16, tag="mk")
            nc.gpsimd.dma_start(mk[:], masks[:])
            ma, mb = mk[:, 0], mk[:, 1]

            # ---- gathers (SWDGE, program order = arrival order) --------
            GP = singles.tile([BC, H], BF16, tag="gp_rows")
            nc.gpsimd.indirect_dma_start(
                out=GP[:], out_offset=None, in_=bert2d,
                in_offset=bass.IndirectOffsetOnAxis(ap=idx[0:BC, 8:9], axis=0),
            )

            def gather_span(col0, tag):
                tiles = []
                for c in range(NCH):
                    g = singles.tile([128, H], BF16, tag=f"{tag}{c}")
                    nc.gpsimd.indirect_dma_start(
                        out=g[:], out_offset=None,
                        in_=bert2d,
                        in_offset=bass.IndirectOffsetOnAxis(
                            ap=idx[:, col0 + c:col0 + c + 1], axis=0),
                    )
                    tiles.append(g)
                return tiles

            GA = gather_span(0, "ga")

            # bias pack broadcast (needed first at l1p epilogue)
            bias_t = singles.tile([BC, 8, 1024], BF16, tag="bias")
            nc.gpsimd.dma_start(bias_t[:], _bcast_rows(biases[:], BC))
            rep = {
                "bp1": bias_t[:, 0, :], "gp": bias_t[:, 1, :],
                "betap": bias_t[:, 2, :], "be1": bias_t[:, 3, :],
                "ge": bias_t[:, 4, :], "betae": bias_t[:, 5, :],
                "bp2": bias_t[:, 6, 0:HH], "be2": bias_t[:, 6, HH:2 * HH],
                "bl": bias_t[:, 7, 0:LH], "bc": bias_t[:, 7, LH:LH + NOUT],
            }

            GB = gather_span(NCH, "gb")

            # absorb the bias DMA lane into the DVE clock
            dve_scratch = singles.tile([1, 16], F32, tag="dve_scratch")
            nc.vector.tensor_copy(dve_scratch[0:1, 0:1], bias_t[0:1, 0, 0:1])

            # ---- weight preload: consumption order, alternating HWDGE --
            wsb = {}
            wload = {}          # name -> list of (k_start, dma_inst)
            eng_i = [0]

            def preload(name):
                _, K, N, kk = next(s for s in MAT_SPECS if s[0] == name)
                nk = K // 128
                t = singles.tile([128, nk, N], BF16, tag=f"w_{name}")
                wsb[name] = t
                wload[name] = []
                for k0 in range(0, nk, kk):
                    eng = nc.sync if eng_i[0] % 2 == 0 else nc.scalar
                    eng_i[0] += 1
                    ld = eng.dma_start(t[:, k0:k0 + kk, :],
                                       wd[name][:, k0:k0 + kk, :])
                    wload[name].append((k0, ld))

            for name, _, _, _ in MAT_SPECS:
                preload(name)

            def load_for(name, k):
                """DMA inst that delivers k-chunk k of weight `name`."""
                _, _, _, kk = next(s for s in MAT_SPECS if s[0] == name)
                return wload[name][k // kk][1]

            pe_observe(ma[0:32, 0, 0:32], "ma")
            pe_observe(mb[0:32, 0, 0:32], "mb")

            # ---- span features: S = M.T @ G  -> [96, H] ----------------
            def span_feats(m_tile, g_tiles, tag):
                ps = [pshare.tile([96, 512], F32, tag="share", name=f"ps_{tag}{h}")
                      for h in range(2)]
                for c in range(NCH):
                    for h in range(2):
                        nc.tensor.matmul(
                            ps[h][:],
                            lhsT=m_tile[:, c, :],
                            rhs=g_tiles[c][:, h * 512:(h + 1) * 512],
                            start=(c == 0), stop=(c == NCH - 1),
                        )
                sb = singles.tile([96, H], BF16, tag="sf")
                for h in range(2):
                    nc.vector.tensor_copy(sb[:, h * 512:(h + 1) * 512], ps[h][:])
                return sb

            SA = span_feats(ma, GA, "a")

            # transpose span feats -> [128, 8, 96] per side (bf16 in/out)
            def transpose_feats(src, tag):
                dst = singles.tile([128, 8, 96], BF16, tag=f"t_{tag}")
                cp = None
                for h in range(8):
                    pt = pshare.tile([128, 96], BF16, tag="share", name="pt96")
                    nc.tensor.transpose(
                        pt[:], src[:, h * 128:(h + 1) * 128], ident96[:])
                    cp = nc.vector.tensor_copy(dst[:, h, :], pt[:])
                return dst, cp

            AT, AT_cp = transpose_feats(SA, "a")
            SB = span_feats(mb, GB, "b")
            BT, BT_cp = transpose_feats(SB, "b")

            # pron rows transposed -> [128, 8, 32]
            pe_observe(GP[0:32, 0:32], "gp_lane")
            PT = singles.tile([128, 8, BC], BF16, tag="ptron")
            PT_cp = None
            for h in range(8):
                pt = pshare.tile([128, 96], BF16, tag="share", name="pt32")
                pt = pt[:, :BC]
                nc.tensor.transpose(
                    pt[:], GP[:, h * 128:(h + 1) * 128], ident32[:])
                PT_cp = nc.vector.tensor_copy(PT[:, h, :], pt[:])

            # transpose a batch-major bf16 [BC, n*128] activation -> [128, n, BC]
            def transpose_act(src, n, tag):
                dst = acts.tile([128, n, BC], BF16, tag=f"tact_{tag}")
                cp = None
                for h in range(n):
                    pt = pshare.tile([128, 96], BF16, tag="share", name="pt32")
                    pt = pt[:, :BC]
                    nc.tensor.transpose(
                        pt[:], src[:, h * 128:(h + 1) * 128], ident32[:])
                    cp = nc.vector.tensor_copy(dst[:, h, :], pt[:])
                return dst, cp

            # matmul over preloaded weights: lhsT chunks [128, m] bf16
            last_mm = [None]

            def sb_matmul(psum_ap, lhsT_chunks, name, ktiles, n_out,
                          lhsT_deps=()):
                for k in range(ktiles):
                    dr_e = None
                    if k == 0:
                        dr_e = engine_absorb(nc.tensor, *lhsT_deps,
                                             last_mm[0])
                    mm = None
                    for h in range(0, n_out, 512):
                        hi = min(h + 512, n_out)
                        mm = nc.tensor.matmul(
                            psum_ap[:, h:hi],
                            lhsT=lhsT_chunks(k),
                            rhs=wsb[name][:, k, h:hi],
                            start=(k == 0), stop=(k == ktiles - 1),
                        )
                        order_after(mm, dr_e)
                    last_mm[0] = mm

            # LayerNorm + affine + leaky-relu epilogue -> bf16 [BC, n]
            def ln_leaky(psum_t, bias_ap, g_ap, beta_ap, n, tag):
                x = acts.tile([BC, n], F32, tag="ln_x")
                nc.vector.tensor_add(x[:], psum_t[:], bias_ap)
                nsub = n // 512
                stats = acts.tile([BC, nsub, 6], F32, tag="ln_st")
                xv = x[:].rearrange("p (s f) -> p s f", f=512)
                for s in range(nsub):
                    nc.vector.bn_stats(out=stats[:, s, :], in_=xv[:, s, :])
                mv = acts.tile([BC, 2], F32, tag="ln_mv")
                nc.vector.bn_aggr(out=mv[:], in_=stats[:])
                std = acts.tile([BC, 1], F32, tag="ln_sd")
                nc.scalar.activation(
                    out=std[:], in_=mv[:, 1:2],
                    func=mybir.ActivationFunctionType.Sqrt,
                    bias=eps_t[:], scale=1.0)
                rstd = acts.tile([BC, 1], F32, tag="ln_rs")
                nc.vector.reciprocal(out=rstd[:], in_=std[:])
                nc.vector.tensor_scalar(
                    out=x[:], in0=x[:], scalar1=mv[:, 0:1], scalar2=rstd[:],
                    op0=mybir.AluOpType.subtract, op1=mybir.AluOpType.mult)
                nc.vector.tensor_mul(x[:], x[:], g_ap)
                nc.vector.tensor_add(x[:], x[:], beta_ap)
                # leaky relu: max(x,0) + 0.01*min(x,0), emitted bf16
                pos = acts.tile([BC, n], F32, tag="ln_pos")
                nc.vector.tensor_scalar_max(pos[:], x[:], 0.0)
                nc.vector.tensor_scalar(
                    out=x[:], in0=x[:], scalar1=0.0, scalar2=0.01,
                    op0=mybir.AluOpType.min, op1=mybir.AluOpType.mult)
                xb = acts.tile([BC, n], BF16, tag="ln_out")
                nc.vector.tensor_add(xb[:], x[:], pos[:])
                return xb

            # ---- pron branch layer 1 -----------------------------------
            ps1p = pbig.tile([BC, H], F32, tag="psA", name="ps1p")
            sb_matmul(ps1p, lambda k: PT[:, k, :], "Wp1", 8, H,
                      lhsT_deps=(PT_cp,))
            X1p = ln_leaky(ps1p, rep["bp1"], rep["gp"], rep["betap"], H, "p")
            X1pT, X1pT_cp = transpose_act(X1p, 8, "x1p")

            # ---- ent branch layer 1 ------------------------------------
            def ent_chunk(k):
                blk, h = divmod(k, 8)
                side = AT if blk < 3 else BT
                b = blk % 3
                return side[:, h, b * 32:(b + 1) * 32]

            ps1e = pbig.tile([BC, H], F32, tag="psB", name="ps1e")
            # absorb the We1 tail's DMA lane before the group; per-k waits
            # ride on each matmul via the framework.
            sb_matmul(ps1e, ent_chunk, "We1", 48, H,
                      lhsT_deps=(AT_cp, BT_cp))
            X1e = ln_leaky(ps1e, rep["be1"], rep["ge"], rep["betae"], H, "e")
            X1eT, X1eT_cp = transpose_act(X1e, 8, "x1e")

            # ---- layer 2 (both branches into one concat tile) ----------
            ps2 = pbig.tile([BC, 2 * HH], F32, tag="psA", name="ps2")
            sb_matmul(ps2[:, 0:HH], lambda k: X1pT[:, k, :], "Wp2", 8, HH,
                      lhsT_deps=(X1pT_cp,))
            sb_matmul(ps2[:, HH:2 * HH], lambda k: X1eT[:, k, :], "We2", 8, HH,
                      lhsT_deps=(X1eT_cp,))
            XC = acts.tile([BC, 2 * HH], BF16, tag="xc")
            nc.vector.tensor_add(XC[:, 0:HH], ps2[:, 0:HH], rep["bp2"])
            nc.vector.tensor_add(XC[:, HH:], ps2[:, HH:], rep["be2"])

            XCT, XCT_cp = transpose_act(XC, 8, "xc")

            # ---- final hidden + exact gelu -----------------------------
            ps3 = pshare.tile([BC, LH], F32, tag="share", name="ps3")
            sb_matmul(ps3, lambda k: XCT[:, k, :], "Wl", 8, LH,
                      lhsT_deps=(XCT_cp,))
            g = acts.tile([BC, LH], F32, tag="g")
            g_add = nc.vector.tensor_add(g[:], ps3[:], rep["bl"])
            erf = acts.tile([BC, LH], F32, tag="erf")
            erf_act = nc.scalar.activation(
                out=erf[:], in_=g[:],
                func=mybir.ActivationFunctionType.Erf,
                bias=0.0, scale=float(1.0 / np.sqrt(2.0)))
            ge_t = acts.tile([BC, LH], F32, tag="ge_t")
            dr_g = engine_absorb(nc.vector, g_add, erf_act)
            gm = nc.vector.tensor_mul(ge_t[:], g[:], erf[:])
            order_after(gm, dr_g)
            nc.vector.tensor_add(ge_t[:], ge_t[:], g[:])
            geb = acts.tile([BC, LH], BF16, tag="geb")
            nc.vector.tensor_scalar_mul(geb[:], ge_t[:], 0.5)

            GT, GT_cp = transpose_act(geb, 4, "gt")

            # ---- logits -------------------------------------------------
            ps4 = pshare.tile([BC, NOUT], F32, tag="share", name="ps4")
            dr_wc = engine_absorb(nc.tensor, GT_cp, load_for("Wc", 3),
                                  last_mm[0])
            for k in range(4):
                mm = nc.tensor.matmul(
                    ps4[:], lhsT=GT[:, k, :], rhs=wsb["Wc"][:, k, :],
                    start=(k == 0), stop=(k == 3))
                order_after(mm, dr_wc)
            res = acts.tile([BC, NOUT], F32, tag="res")
            res_add = nc.vector.tensor_add(res[:], ps4[:], rep["bc"])
            engine_absorb(nc.sync, res_add)
            nc.sync.dma_start(out[:], res[:])

    import os
    if not os.environ.get('SKIP_PRUNE'):
        _prune_covered_waits(nc)
    nc.finalize()
    return nc


def _prune_covered_waits(nc):
    """Walrus on this toolchain accepts only one sync-wait on most
    instructions (Drain accepts many).  Within a basic block, same-engine
    instructions execute in order, so a wait already issued by an earlier
    same-engine instruction (e.g. an absorber drain) is redundant on a
    later one and can be dropped."""
    for fn in nc.m.functions:
        for blk in fn.blocks:
            insert = []
            for pos, inst in enumerate(blk.instructions):
                si = inst.sync_info
                if (inst.opcode == "Drain" and si and si.on_wait
                        and len(si.on_wait) > 1):
                    extra = list(si.on_wait[:-1])
                    si.on_wait = [si.on_wait[-1]]
                    insert.append((pos, inst, extra))
            for pos, inst, extra in reversed(insert):
                new_insts = []
                for w in extra:
                    d = mybir.InstDrain(
                        name=nc.get_next_instruction_name(),
                        ins=[], outs=[], bass_is_fusable=False)
                    d.engine = inst.engine
                    d.sync_info = mybir.SyncInfo(on_wait=[w], on_update=[])
                    nc.register_instruction(d)
                    new_insts.append(d)
                blk.instructions[pos:pos] = new_insts

    PRUNABLE = ("DMAHW", "DMASW", "PE_", "DVE_", "Pool_", "Activation_",
                "SP_")

    def prunable(w):
        return (getattr(w, "wait_mode", None) == "sem-ge-imm"
                and w.ant_name.startswith(PRUNABLE))

    for fn in nc.m.functions:
        for blk in fn.blocks:
            observed = {}

            for inst in blk.instructions:
                si = inst.sync_info
                eng = str(inst.engine)
                if si and si.on_wait:
                    kept = []
                    for w in si.on_wait:
                        if prunable(w) and (
                                observed.get((eng, w.ant_name), -1)
                                >= w.wait_value):
                            continue
                        kept.append(w)
                    for w in si.on_wait:
                        key = (eng, w.ant_name)
                        if prunable(w):
                            if observed.get(key, -1) < w.wait_value:
                                observed[key] = w.wait_value
                    if len(kept) != len(si.on_wait):
                        si.on_wait = kept

    # Any non-Drain instruction still holding >1 wait: move all but the last
    # into a chain of single-wait Drains inserted just before it (same
    # engine, so ordering is preserved and walrus sees one wait everywhere).
    for fn in nc.m.functions:
        for blk in fn.blocks:
            insert = []
            for pos, inst in enumerate(blk.instructions):
                si = inst.sync_info
                if (inst.opcode != "Drain" and si and si.on_wait
                        and len(si.on_wait) > 1):
                    extra = list(si.on_wait[:-1])
                    si.on_wait = [si.on_wait[-1]]
                    insert.append((pos, inst, extra))
            for pos, inst, extra in reversed(insert):
                new_insts = []
                for w in extra:
                    d = mybir.InstDrain(
                        name=nc.get_next_instruction_name(),
                        ins=[], outs=[], bass_is_fusable=False)
                    d.engine = inst.engine
                    d.sync_info = mybir.SyncInfo(on_wait=[w], on_update=[])
                    nc.register_instruction(d)
                    new_insts.append(d)
                blk.instructions[pos:pos] = new_insts


_PROGRAM = None


def _get_program():
    global _PROGRAM
    if _PROGRAM is None:
        _PROGRAM = _build_program()
    return _PROGRAM


def make_in_maps(**inputs):
    """Shard full inputs into per-core input maps (host-side prep)."""
    BF = ml_dtypes.bfloat16
    bert = np.ascontiguousarray(
        np.asarray(inputs["bert_outputs"], dtype=np.float32)).astype(BF)
    offsets = np.asarray(inputs["offsets"], dtype=np.int32)

    wT = {}
    for name, K, N, _ in MAT_SPECS:
        W = np.asarray(inputs[name], dtype=np.float32).astype(BF)
        wT[f"{name}T"] = np.ascontiguousarray(
            W.reshape(K // 128, 128, N).transpose(1, 0, 2))

    bias_pack = np.zeros((8, 1024), np.float32)  # cast to BF below
    for r, names in enumerate(BIAS_ROWS):
        col = 0
        for nm in names:
            v = np.asarray(inputs[nm], dtype=np.float32)
            bias_pack[r, col:col + v.shape[0]] = v
            col += v.shape[0]

    in_maps = []
    for c in range(NCORES):
        ob = offsets[c * BC:(c + 1) * BC]
        m = {"bert": bert[c * BC:(c + 1) * BC]}

        def span_desc(s, e):
            ln = (e - s).astype(np.int64)          # [BC], 1..15
            j = np.arange(LSPAN)
            rows = (np.arange(BC) * S)[:, None] + s[:, None] + j[None, :]
            idx = np.zeros(KPAD, np.int32)
            idx[:KROWS] = rows.reshape(-1)
            M = np.zeros((KPAD, 3 * BC), np.float32)
            for b in range(BC):
                base = b * LSPAN
                M[base, b] = 1.0                          # first
                M[base + ln[b] - 1, BC + b] = 1.0         # last
                M[base:base + ln[b], 2 * BC + b] = 1.0 / ln[b]  # mean
            return (idx.reshape(NCH, 128).T.copy(),
                    np.ascontiguousarray(
                        M.reshape(NCH, 128, 3 * BC).transpose(1, 0, 2)))

        iA, MA = span_desc(ob[:, 0], ob[:, 1])
        iB, MB = span_desc(ob[:, 2], ob[:, 3])
        idx_pack = np.zeros((128, 9), np.int32)
        idx_pack[:, 0:4] = iA
        idx_pack[:, 4:8] = iB
        idx_pack[:BC, 8] = np.arange(BC, dtype=np.int32) * S + ob[:, 4]
        m["idxs"] = idx_pack
        m["masks"] = np.ascontiguousarray(
            np.stack([MA, MB], axis=1)).astype(BF)
        m["biases"] = bias_pack.astype(BF)
        m.update(wT)
        in_maps.append(m)
    return in_maps


def run(in_maps, **kwargs):
    nc = _get_program()
    return run_bass_kernel_spmd(nc, in_maps, core_ids=list(range(NCORES)), **kwargs)


def kernel(**inputs):
    res = run(make_in_maps(**inputs))
    return np.concatenate([res.results[c]["out"] for c in range(NCORES)],
                          axis=0).astype(np.float32)
